# revision 1
# baseline (speedup 1.0000x reference)
"""Causal sparse (sliding-window) attention for Trainium2, 8 NeuronCores.

Sharding: tensor-parallel over heads (16 heads -> 2 per core).  Each core
computes the qkv projection for its 2 heads (w_qkv column-parallel), windowed
causal attention, and a partial output projection (w_out row-parallel).
The host sums the 8 partial outputs.

Layout strategy (everything lives transposed so the PE contracts naturally):
  xT [D, L] streamed per 512-column chunk
  qT/kT/vT [hd (2 heads packed on partitions), L] from the QKV matmuls
  RoPE: rotate-half is a [128x128] permutation matmul; combine on DVE
  scoresT [k, q] computed directly (k as lhsT, q as rhs), 2 heads row-packed
  softmax: exp only (scores are small; no max subtraction), masks are
  multiplicative 0/1 on the two partial 128x128 blocks per query tile
  AV: v augmented with a ones-column -> denominator lands in the psum,
  normalization fused into the psum->sbuf copy
  out projection: ctxT [128, L] directly as lhsT, w_out rows as rhs
All matmuls in float32r (full PE rate at moving dim >= 256).  Emission is
interleaved per 512-query superblock so all engines pipeline across phases.
"""
import numpy as np

import concourse.bacc as bacc
import concourse.tile as tile
import concourse.mybir as mybir
from concourse.bass_utils import run_bass_kernel_spmd

F32 = mybir.dt.float32
F32R = mybir.dt.float32r
BF16 = mybir.dt.bfloat16

D = 1024
L = 4096
HD = 64
N_CORES = 8
WINDOW = 512
ROPE_BASE = 10000.0
NSB = L // 512          # superblocks of 512 queries
NQB = L // 128          # 128-query blocks


def _attn_plan(sb):
    """Per-superblock key-block plan: (abs key block, lo, hi, diag_qi, far_qi).
    lo/hi bound the valid query blocks (in 0..4) for that key block; diag/far
    mark which query block needs the triangular partial mask."""
    if sb == 0:
        return [(kb, kb, 4, kb, None) for kb in range(4)]
    plan = []
    for ki in (4, 0, 1, 2, 3, 5, 6, 7):   # ki=4 first: full span, start=True
        plan.append((sb * 4 - 4 + ki, max(0, ki - 4), min(3, ki) + 1,
                     ki - 4 if ki >= 4 else None, ki if ki <= 3 else None))
    return plan


_TAGS = {}


def _tag(ret, label):
    try:
        _TAGS[ret.ins.name] = label
    except Exception:
        pass
    return ret


def _build_nc(phases=("qkv", "attn", "out"), iters=1):
    _TAGS.clear()
    nc = bacc.Bacc(None, target_bir_lowering=False)

    xT = nc.dram_tensor("xT", [D, L], F32R, kind="ExternalInput")
    wl = nc.dram_tensor("wl", [D, 384], F32R, kind="ExternalInput")
    wo = nc.dram_tensor("wo", [128, D], F32R, kind="ExternalInput")
    p2 = nc.dram_tensor("p2", [128, 128], F32R, kind="ExternalInput")
    cs = nc.dram_tensor("cs", [128, L], F32, kind="ExternalInput")
    sn = nc.dram_tensor("sn", [128, L], F32, kind="ExternalInput")
    md = nc.dram_tensor("md", [128, 128], F32, kind="ExternalInput")
    mf = nc.dram_tensor("mf", [128, 128], F32, kind="ExternalInput")
    ident = nc.dram_tensor("ident", [128, 128], F32R, kind="ExternalInput")
    onesd = nc.dram_tensor("onesd", [128, 32], BF16, kind="ExternalInput")
    onesr = nc.dram_tensor("onesr", [1, 64], F32R, kind="ExternalInput")
    po = nc.dram_tensor("po", [L, D], F32, kind="ExternalOutput")

    xT3 = xT.rearrange("(ko ki) l -> ki ko l", ki=128)   # [128, 8, L]
    wl3 = wl.rearrange("(ko ki) m -> ki ko m", ki=128)   # [128, 8, 384]

    with tile.TileContext(nc) as tc:
        with tc.tile_pool(name="singles", bufs=1) as singles, \
             tc.tile_pool(name="work", bufs=2) as work, \
             tc.tile_pool(name="ptp", bufs=6) as ptp, \
             tc.tile_pool(name="outp", bufs=6) as outp, \
             tc.tile_pool(name="ps", bufs=4, space="PSUM") as ps, \
             tc.tile_pool(name="pst", bufs=2, space="PSUM") as pst:

            w_sb = singles.tile([128, 8, 384], F32R)
            nc.sync.dma_start(w_sb[:], wl3[:])
            p2_sb = singles.tile([128, 128], F32R)
            nc.sync.dma_start(p2_sb[:], p2[:])
            id_sb = singles.tile([128, 128], F32R)
            nc.sync.dma_start(id_sb[:], ident[:])
            onesr_sb = singles.tile([1, 64], F32R)
            nc.sync.dma_start(onesr_sb[:], onesr[:])
            wo_sb = singles.tile([128, D], F32R)
            cs_sb = singles.tile([128, L], F32)
            sn_sb = singles.tile([128, L], F32)
            md_sb = singles.tile([128, 128], F32)
            mf_sb = singles.tile([128, 128], F32)

            qrot_sb = singles.tile([128, L], F32R)
            krot_sb = singles.tile([128, L], F32R)
            ctxT_sb = singles.tile([128, L], F32R)
            # v natural layout per 128-key block: [h0 v(64) | 1 | h1 v(64) | 1]
            v_sb = singles.tile([128, NQB, 130], BF16)

            # rope tables are read by chunk 0 already: emit before the loop,
            # but on the gpsimd queue so they don't head-block the x stream
            nc.gpsimd.dma_start(cs_sb[:], cs[:])
            nc.gpsimd.dma_start(sn_sb[:], sn[:])

            def emit_const_dmas():
                # first read happens in attention(0), emitted one slot later
                nc.gpsimd.dma_start(wo_sb[:], wo[:])
                nc.gpsimd.dma_start(md_sb[:], md[:])
                nc.gpsimd.dma_start(mf_sb[:], mf[:])
                nc.gpsimd.dma_start(v_sb[:, :, 64:65], onesd[:, :, None])
                nc.gpsimd.dma_start(v_sb[:, :, 129:130], onesd[:, :, None])

            def emit_qkv_chunk(n):
                span = slice(n * 512, (n + 1) * 512)
                xt = work.tile([128, 8, 512], F32R, tag="xt")
                nc.sync.dma_start(xt[:], xT3[:, :, span])

                # all three projections back-to-back on PE, copies chase on
                # ACT, then the rotate-half matmuls (their ACT inputs are
                # ready by then), then the v transposes.
                prj = []
                for m in range(3):
                    psq = ps.tile([128, 512], F32, tag="mm")
                    for k8 in range(8):
                        _tag(nc.tensor.matmul(
                            psq[:], w_sb[:, k8, m * 128:(m + 1) * 128],
                            xt[:, k8, :], start=(k8 == 0), stop=(k8 == 7)),
                            f"qkvmm n{n} m{m} k{k8}")
                    raw = work.tile([128, 512], F32R,
                                    tag="qkraw" if m < 2 else "vraw")
                    nc.scalar.copy(raw[:], psq[:])
                    prj.append((psq, raw))

                for m, dst in ((0, qrot_sb), (1, krot_sb)):
                    psq, raw = prj[m]
                    psr = ps.tile([128, 512], F32, tag="mm")
                    _tag(nc.tensor.matmul(psr[:], p2_sb[:], raw[:],
                                     start=True, stop=True), f"rotmm n{n} m{m}")
                    qc = work.tile([128, 512], F32, tag="qc")
                    nc.vector.tensor_tensor(qc[:], psq[:], cs_sb[:, span],
                                            mybir.AluOpType.mult)
                    qs = work.tile([128, 512], F32, tag="qs")
                    nc.vector.tensor_tensor(qs[:], psr[:], sn_sb[:, span],
                                            mybir.AluOpType.mult)
                    nc.vector.tensor_tensor(dst[:, span], qc[:], qs[:],
                                            mybir.AluOpType.add)

                vraw = prj[2][1]
                for j in range(4):
                    blk = n * 4 + j
                    tp = pst.tile([128, 128], F32R, tag="tp")
                    _tag(nc.tensor.transpose(tp[:], vraw[:, j * 128:(j + 1) * 128],
                                        id_sb[:]), f"vtp n{n} j{j}")
                    nc.vector.tensor_copy(v_sb[:, blk, 0:64], tp[:, 0:64])
                    nc.vector.tensor_copy(v_sb[:, blk, 65:129], tp[:, 64:128])

            def emit_attention_sb(sb):
                plan = _attn_plan(sb)
                n_av = len(plan)
                ctxs = [ps.tile([128, 512], F32, tag="ctx", bufs=2, name=f"ctx{h}")
                        for h in range(2)]

                def emit_score(h, idx):
                    kb, lo, hi, diag_qi, far_qi = plan[idx]
                    hp = slice(h * 64, (h + 1) * 64)
                    cspan = slice(lo * 128, hi * 128)
                    qspan = slice(sb * 512 + lo * 128, sb * 512 + hi * 128)
                    scp = ps.tile([128, 512], F32, tag="mm", name="scp")
                    _tag(nc.tensor.matmul(
                        scp[:, cspan],
                        krot_sb[hp, kb * 128:(kb + 1) * 128],
                        qrot_sb[hp, qspan],
                        start=True, stop=True,
                        tile_position=(h * 64, 0)), f"scmm sb{sb} h{h} i{idx}")
                    pt = ptp.tile([128, 512], BF16, tag="pt", name="pt")
                    nc.scalar.activation(
                        pt[:, cspan], scp[:, cspan],
                        mybir.ActivationFunctionType.Exp, scale=0.125)
                    if far_qi is not None:
                        fsp = slice(far_qi * 128, (far_qi + 1) * 128)
                        nc.vector.tensor_tensor(pt[:, fsp], pt[:, fsp],
                                                mf_sb[:],
                                                mybir.AluOpType.mult)
                    if diag_qi is not None:
                        dsp = slice(diag_qi * 128, (diag_qi + 1) * 128)
                        nc.vector.tensor_tensor(pt[:, dsp], pt[:, dsp],
                                                md_sb[:],
                                                mybir.AluOpType.mult)
                    return pt

                def emit_av(h, idx, pt):
                    kb, lo, hi, _, _ = plan[idx]
                    cspan = slice(lo * 128, hi * 128)
                    _tag(nc.tensor.matmul(
                        ctxs[h][0:65, cspan],
                        v_sb[:, kb, h * 65:(h + 1) * 65],
                        pt[:, cspan],
                        start=(idx == 0), stop=(idx == n_av - 1),
                        skip_group_check=True), f"avmm sb{sb} h{h} i{idx}")

                # software pipeline: AV lags the score/exp/mask chain by one
                # key block, both heads interleaved, so the PE never waits on
                # the ACT+POOL round trip.
                pts = {}
                for idx in range(n_av):
                    for h in range(2):
                        pts[(h, idx)] = emit_score(h, idx)
                    if idx > 1:
                        for h in range(2):
                            emit_av(h, idx - 2, pts.pop((h, idx - 2)))
                for idx in (n_av - 2, n_av - 1):
                    for h in range(2):
                        emit_av(h, idx, pts.pop((h, idx)))

                sspan = slice(sb * 512, (sb + 1) * 512)
                for h in range(2):
                    hp = slice(h * 64, (h + 1) * 64)
                    rt = work.tile([1, 512], F32, tag="rt")
                    nc.vector.reciprocal(rt[:], ctxs[h][64:65, :])
                    rb = work.tile([64, 512], F32, tag="rb")
                    nc.gpsimd.partition_broadcast(rb[:], rt[:])
                    nc.vector.tensor_tensor(ctxT_sb[hp, sspan],
                                            ctxs[h][0:64, :],
                                            rb[:], mybir.AluOpType.mult)

            def emit_outproj_sb(sb):
                for ti, t in enumerate(range(sb * 4, sb * 4 + 4)):
                    for nn in range(2):
                        op = ps.tile([128, 512], F32, tag="mm")
                        _tag(nc.tensor.matmul(
                            op[:], ctxT_sb[:, t * 128:(t + 1) * 128],
                            wo_sb[:, nn * 512:(nn + 1) * 512],
                            start=True, stop=True), f"outmm t{t} n{nn}")
                        osb = outp.tile([128, 512], F32, tag="ob")
                        if (ti * 2 + nn) % 2 == 0:
                            nc.scalar.copy(osb[:], op[:])
                        else:
                            nc.vector.tensor_copy(osb[:], op[:])
                        nc.sync.dma_start(
                            po[t * 128:(t + 1) * 128,
                               nn * 512:(nn + 1) * 512], osb[:])

            def emit_body():
                for n in range(NSB + 2):
                    if n < NSB and "qkv" in phases:
                        emit_qkv_chunk(n)
                    if n == 0:
                        emit_const_dmas()
                    if 1 <= n <= NSB and "attn" in phases:
                        emit_attention_sb(n - 1)
                    if n >= 2 and "out" in phases:
                        emit_outproj_sb(n - 2)

            if iters == 1:
                emit_body()
            else:
                with tc.For_i(0, iters, 1):
                    emit_body()
    nc.finalize()
    return nc


def _host_constants():
    # RoPE tables, transposed + duplicated for the two packed head halves
    inv_freq = (1.0 / (ROPE_BASE ** (np.arange(0, HD, 2, dtype=np.float32)
                                     / np.float32(HD)))).astype(np.float32)
    pos = np.arange(L, dtype=np.float32)
    freqs = pos[:, None] * inv_freq[None, :]            # [L, 32]
    cos = np.repeat(np.cos(freqs), 2, axis=-1).astype(np.float32)  # [L, 64]
    sin = np.repeat(np.sin(freqs), 2, axis=-1).astype(np.float32)
    cs = np.ascontiguousarray(np.vstack([cos.T, cos.T]))  # [128, L]
    sn = np.ascontiguousarray(np.vstack([sin.T, sin.T]))

    # rotate-half as a column-space permutation: rh(q) = q @ Pc
    pc = np.zeros((HD, HD), np.float32)
    for m in range(HD // 2):
        pc[2 * m + 1, 2 * m] = -1.0
        pc[2 * m, 2 * m + 1] = 1.0
    p2 = np.zeros((128, 128), np.float32)
    p2[:64, :64] = pc
    p2[64:, 64:] = pc

    k_idx = np.arange(128)[:, None]
    q_idx = np.arange(128)[None, :]
    md = (k_idx <= q_idx).astype(np.float32)   # diag block: valid k <= q
    mf = (k_idx > q_idx).astype(np.float32)    # far block: valid k > q
    ident = np.eye(128, dtype=np.float32)
    import ml_dtypes
    onesd = np.ones((128, 32), ml_dtypes.bfloat16)
    onesr = np.ones((1, 64), np.float32)
    return cs, sn, p2, md, mf, ident, onesd, onesr


_NC_CACHE = {}


def kernel(x, w_qkv, w_out):
    x = np.asarray(x, np.float32)
    w_qkv = np.asarray(w_qkv, np.float32)
    w_out = np.asarray(w_out, np.float32)
    B = x.shape[0]
    assert x.shape == (B, L, D) and B == 1

    if "nc" not in _NC_CACHE:
        _NC_CACHE["nc"] = _build_nc()
    nc = _NC_CACHE["nc"]

    xT = np.ascontiguousarray(x[0].T)                  # [D, L]
    cs, sn, p2, md, mf, ident, onesd, onesr = _host_constants()

    in_maps = []
    for c in range(N_CORES):
        h0 = 2 * c
        col = slice(h0 * HD, (h0 + 2) * HD)
        wl = np.ascontiguousarray(np.concatenate(
            [w_qkv[:, 0 * D:1 * D][:, col],
             w_qkv[:, 1 * D:2 * D][:, col],
             w_qkv[:, 2 * D:3 * D][:, col]], axis=1))  # [D, 384]
        wo = np.ascontiguousarray(w_out[h0 * HD:(h0 + 2) * HD, :])  # [128, D]
        in_maps.append({"xT": xT, "wl": wl, "wo": wo, "p2": p2,
                        "cs": cs, "sn": sn, "md": md, "mf": mf,
                        "ident": ident, "onesd": onesd, "onesr": onesr})

    res = run_bass_kernel_spmd(nc, in_maps, core_ids=list(range(N_CORES)))
    out = np.zeros((L, D), np.float64)
    for r in res.results:
        out += r["po"].astype(np.float64)
    return out.astype(np.float32)[None]



# revision 8
# speedup vs baseline: 2.5591x; 2.5591x over previous
"""Causal sparse (sliding-window) attention for Trainium2, 8 NeuronCores.

Sharding: tensor-parallel over heads (16 heads -> 2 per core).  Each core
computes the qkv projection for its 2 heads (w_qkv column-parallel), windowed
causal attention, and a partial output projection (w_out row-parallel).
The host sums the 8 partial outputs.

v2 (this file): everything bf16 end-to-end.
  - All HBM I/O in bf16: x (8MB), partial out (8MB), rope tables, weights.
    Halves both per-core DMA busy and chip-level HBM contention.
  - All matmuls bf16 (1 cycle/row at any moving size; PSUM accumulates f32).
  - Rope combine + masks run on DVE in all-bf16 SBUF mode (2x throughput).
  - Both heads' score matmuls land in one [128,2,512] PSUM tile, so the
    exp is ONE wide ACT op per key block (halves ACT fixed costs); same
    pairing for the out-projection psum -> one wide copy + one wide DMA.
  - qkv psum->sbuf copies moved to the (idle) GpSimd engine.
Layout strategy otherwise identical to v1:
  xT [D, L] streamed per 512-column chunk
  qT/kT/vT [hd (2 heads packed on partitions), L] from the QKV matmuls
  RoPE: rotate-half is a [128x128] permutation matmul; combine on DVE
  scoresT [k, q] computed directly (k as lhsT, q as rhs)
  softmax: exp only (scores are small); masks multiplicative 0/1 bf16
  AV: v augmented with a ones-column -> denominator in the psum
  out projection: ctxT [128, L] as lhsT, w_out rows as rhs
"""
import numpy as np

import concourse.bacc as bacc
import concourse.tile as tile
import concourse.mybir as mybir
from concourse.bass_utils import run_bass_kernel_spmd

F32 = mybir.dt.float32
BF16 = mybir.dt.bfloat16

D = 1024
L = 4096
HD = 64
N_CORES = 8
WINDOW = 512
ROPE_BASE = 10000.0
NSB = L // 512          # superblocks of 512 queries
NQB = L // 128          # 128-query blocks


def _attn_plan(sb):
    """Per-superblock key-block plan: (abs key block, lo, hi, diag_qi, far_qi).
    lo/hi bound the valid query blocks (in 0..4) for that key block; diag/far
    mark which query block needs the triangular partial mask."""
    if sb == 0:
        return [(kb, kb, 4, kb, None) for kb in range(4)]
    plan = []
    for ki in (4, 0, 1, 2, 3, 5, 6, 7):   # ki=4 first: full span, start=True
        plan.append((sb * 4 - 4 + ki, max(0, ki - 4), min(3, ki) + 1,
                     ki - 4 if ki >= 4 else None, ki if ki <= 3 else None))
    return plan


_TAGS = {}


def _tag(ret, label):
    try:
        _TAGS[ret.ins.name] = label
    except Exception:
        pass
    return ret


def _build_nc(phases=("qkv", "attn", "out"), iters=1):
    _TAGS.clear()
    nc = bacc.Bacc(None, target_bir_lowering=False)

    xT = nc.dram_tensor("xT", [D, L], BF16, kind="ExternalInput")
    wl = nc.dram_tensor("wl", [D, 384], BF16, kind="ExternalInput")
    wo = nc.dram_tensor("wo", [128, D], BF16, kind="ExternalInput")
    p2 = nc.dram_tensor("p2", [128, 128], BF16, kind="ExternalInput")
    cs = nc.dram_tensor("cs", [128, L], BF16, kind="ExternalInput")
    sn = nc.dram_tensor("sn", [128, L], BF16, kind="ExternalInput")
    md = nc.dram_tensor("md", [128, 128], BF16, kind="ExternalInput")
    mf = nc.dram_tensor("mf", [128, 128], BF16, kind="ExternalInput")
    ident = nc.dram_tensor("ident", [128, 128], BF16, kind="ExternalInput")
    onesd = nc.dram_tensor("onesd", [128, 32], BF16, kind="ExternalInput")
    po = nc.dram_tensor("po", [L, D], BF16, kind="ExternalOutput")

    xT3 = xT.rearrange("(ko ki) l -> ki ko l", ki=128)   # [128, 8, L]
    wl3 = wl.rearrange("(ko ki) m -> ki ko m", ki=128)   # [128, 8, 384]
    po3 = po.rearrange("l (a b) -> l a b", a=2)          # [L, 2, 512]

    with tile.TileContext(nc) as tc:
        with tc.tile_pool(name="singles", bufs=1) as singles, \
             tc.tile_pool(name="work", bufs=2) as work, \
             tc.tile_pool(name="ptp", bufs=6) as ptp, \
             tc.tile_pool(name="outp", bufs=4) as outp, \
             tc.tile_pool(name="ps", bufs=2, space="PSUM") as ps:

            w_sb = singles.tile([128, 8, 384], BF16)
            nc.sync.dma_start(w_sb[:], wl3[:])
            p2_sb = singles.tile([128, 128], BF16)
            nc.sync.dma_start(p2_sb[:], p2[:])
            id_sb = singles.tile([128, 128], BF16)
            nc.sync.dma_start(id_sb[:], ident[:])
            wo_sb = singles.tile([128, D], BF16)
            cs_sb = singles.tile([128, L], BF16)
            sn_sb = singles.tile([128, L], BF16)
            md_sb = singles.tile([128, 128], BF16)
            mf_sb = singles.tile([128, 128], BF16)

            qrot_sb = singles.tile([128, L], BF16)
            krot_sb = singles.tile([128, L], BF16)
            ctxT_sb = singles.tile([128, L], BF16)
            # v natural layout per 128-key block: [h0 v(64) | 1 | h1 v(64) | 1]
            v_sb = singles.tile([128, NQB, 130], BF16)

            # rope table slices stream just-in-time per chunk (gpsimd queue)
            # so the 2MB of tables never head-blocks the x stream on the bus

            def emit_const_dmas():
                # first read happens in attention(0), emitted one slot later
                nc.gpsimd.dma_start(wo_sb[:], wo[:])
                nc.gpsimd.dma_start(md_sb[:], md[:])
                nc.gpsimd.dma_start(mf_sb[:], mf[:])
                nc.gpsimd.dma_start(v_sb[:, :, 64:65], onesd[:, :, None])
                nc.gpsimd.dma_start(v_sb[:, :, 129:130], onesd[:, :, None])

            def emit_qkv_chunk(n):
                span = slice(n * 512, (n + 1) * 512)
                xt = work.tile([128, 8, 512], BF16, tag="xt")
                nc.sync.dma_start(xt[:], xT3[:, :, span])
                nc.gpsimd.dma_start(cs_sb[:, span], cs[:, span])
                nc.gpsimd.dma_start(sn_sb[:, span], sn[:, span])

                raw = work.tile([128, 3, 512], BF16, tag="raw")
                for m in range(3):
                    psq = ps.tile([128, 512], F32, tag="mm")
                    for k8 in range(8):
                        _tag(nc.tensor.matmul(
                            psq[:], w_sb[:, k8, m * 128:(m + 1) * 128],
                            xt[:, k8, :], start=(k8 == 0), stop=(k8 == 7)),
                            f"qkvmm n{n} m{m} k{k8}")
                    if m < 2:
                        nc.scalar.copy(raw[:, m, :], psq[:])
                    else:
                        nc.vector.tensor_copy(raw[:, m, :], psq[:])

                for m, dst in ((0, qrot_sb), (1, krot_sb)):
                    psr = ps.tile([128, 512], F32, tag="mm")
                    _tag(nc.tensor.matmul(psr[:], p2_sb[:], raw[:, m, :],
                                     start=True, stop=True), f"rotmm n{n} m{m}")
                    rr = work.tile([128, 512], BF16, tag="rr")
                    nc.vector.tensor_copy(rr[:], psr[:])
                    qc = work.tile([128, 512], BF16, tag="qc")
                    nc.vector.tensor_tensor(qc[:], raw[:, m, :], cs_sb[:, span],
                                            mybir.AluOpType.mult)
                    qs = work.tile([128, 512], BF16, tag="qs")
                    nc.vector.tensor_tensor(qs[:], rr[:], sn_sb[:, span],
                                            mybir.AluOpType.mult)
                    nc.vector.tensor_tensor(dst[:, span], qc[:], qs[:],
                                            mybir.AluOpType.add)

                for j in range(4):
                    blk = n * 4 + j
                    tp = ps.tile([128, 128], BF16, tag="mm")
                    _tag(nc.tensor.transpose(tp[:], raw[:, 2, j * 128:(j + 1) * 128],
                                        id_sb[:]), f"vtp n{n} j{j}")
                    nc.vector.tensor_copy(v_sb[:, blk, 0:64], tp[:, 0:64])
                    nc.vector.tensor_copy(v_sb[:, blk, 65:129], tp[:, 64:128])

            def emit_attention_sb(sb):
                plan = _attn_plan(sb)
                n_av = len(plan)
                ctxs = [ps.tile([128, 512], F32, tag="ctx", bufs=2, name=f"ctx{h}")
                        for h in range(2)]

                def emit_score(idx):
                    kb, lo, hi, diag_qi, far_qi = plan[idx]
                    cspan = slice(lo * 128, hi * 128)
                    qspan = slice(sb * 512 + lo * 128, sb * 512 + hi * 128)
                    scp = ps.tile([128, 2, 512], F32, tag="sc", bufs=2,
                                  name="scp")
                    for h in range(2):
                        hp = slice(h * 64, (h + 1) * 64)
                        _tag(nc.tensor.matmul(
                            scp[:, h, cspan],
                            krot_sb[hp, kb * 128:(kb + 1) * 128],
                            qrot_sb[hp, qspan],
                            start=True, stop=True,
                            tile_position=(h * 64, 0)),
                            f"scmm sb{sb} h{h} i{idx}")
                    pt = ptp.tile([128, 2, 512], BF16, tag="pt", name="pt")
                    nc.scalar.activation(
                        pt[:, :, cspan], scp[:, :, cspan],
                        mybir.ActivationFunctionType.Exp, scale=0.125)
                    for h in range(2):
                        if far_qi is not None:
                            fsp = slice(far_qi * 128, (far_qi + 1) * 128)
                            nc.vector.tensor_tensor(pt[:, h, fsp], pt[:, h, fsp],
                                                    mf_sb[:],
                                                    mybir.AluOpType.mult)
                        if diag_qi is not None:
                            dsp = slice(diag_qi * 128, (diag_qi + 1) * 128)
                            nc.vector.tensor_tensor(pt[:, h, dsp], pt[:, h, dsp],
                                                    md_sb[:],
                                                    mybir.AluOpType.mult)
                    return pt

                def emit_av(idx, pt):
                    kb, lo, hi, _, _ = plan[idx]
                    cspan = slice(lo * 128, hi * 128)
                    for h in range(2):
                        _tag(nc.tensor.matmul(
                            ctxs[h][0:65, cspan],
                            v_sb[:, kb, h * 65:(h + 1) * 65],
                            pt[:, h, cspan],
                            start=(idx == 0), stop=(idx == n_av - 1),
                            skip_group_check=True), f"avmm sb{sb} h{h} i{idx}")

                # software pipeline: AV lags the score/exp/mask chain by two
                # key blocks so the PE never waits on the ACT+DVE round trip.
                pts = {}
                for idx in range(n_av):
                    pts[idx] = emit_score(idx)
                    if idx > 1:
                        emit_av(idx - 2, pts.pop(idx - 2))
                for idx in (n_av - 2, n_av - 1):
                    emit_av(idx, pts.pop(idx))

                sspan = slice(sb * 512, (sb + 1) * 512)
                for h in range(2):
                    hp = slice(h * 64, (h + 1) * 64)
                    rt = work.tile([1, 512], F32, tag="rt")
                    nc.vector.reciprocal(rt[:], ctxs[h][64:65, :])
                    rb = work.tile([64, 512], F32, tag="rb")
                    nc.gpsimd.partition_broadcast(rb[:], rt[:])
                    nc.vector.tensor_tensor(ctxT_sb[hp, sspan],
                                            ctxs[h][0:64, :],
                                            rb[:], mybir.AluOpType.mult)

            def emit_outproj_sb(sb):
                for ti, t in enumerate(range(sb * 4, sb * 4 + 4)):
                    op = ps.tile([128, 2, 512], F32, tag="sc", bufs=2,
                                 name="op")
                    for nn in range(2):
                        _tag(nc.tensor.matmul(
                            op[:, nn, :], ctxT_sb[:, t * 128:(t + 1) * 128],
                            wo_sb[:, nn * 512:(nn + 1) * 512],
                            start=True, stop=True), f"outmm t{t} n{nn}")
                    osb = outp.tile([128, 2, 512], BF16, tag="ob")
                    if ti % 2 == 0:
                        nc.scalar.copy(osb[:], op[:])
                    else:
                        nc.vector.tensor_copy(osb[:], op[:])
                    nc.sync.dma_start(po3[t * 128:(t + 1) * 128], osb[:])

            def emit_body():
                for n in range(NSB + 2):
                    if n < NSB and "qkv" in phases:
                        emit_qkv_chunk(n)
                    if n == 0:
                        emit_const_dmas()
                    if 1 <= n <= NSB and "attn" in phases:
                        emit_attention_sb(n - 1)
                    if n >= 2 and "out" in phases:
                        emit_outproj_sb(n - 2)

            if iters == 1:
                emit_body()
            else:
                with tc.For_i(0, iters, 1):
                    emit_body()
    nc.finalize()
    return nc


def _host_constants():
    import ml_dtypes
    # RoPE tables, transposed + duplicated for the two packed head halves
    inv_freq = (1.0 / (ROPE_BASE ** (np.arange(0, HD, 2, dtype=np.float32)
                                     / np.float32(HD)))).astype(np.float32)
    pos = np.arange(L, dtype=np.float32)
    freqs = pos[:, None] * inv_freq[None, :]            # [L, 32]
    cos = np.repeat(np.cos(freqs), 2, axis=-1).astype(np.float32)  # [L, 64]
    sin = np.repeat(np.sin(freqs), 2, axis=-1).astype(np.float32)
    bf = ml_dtypes.bfloat16
    cs = np.ascontiguousarray(np.vstack([cos.T, cos.T])).astype(bf)  # [128, L]
    sn = np.ascontiguousarray(np.vstack([sin.T, sin.T])).astype(bf)

    # rotate-half as a column-space permutation: rh(q) = q @ Pc
    pc = np.zeros((HD, HD), np.float32)
    for m in range(HD // 2):
        pc[2 * m + 1, 2 * m] = -1.0
        pc[2 * m, 2 * m + 1] = 1.0
    p2 = np.zeros((128, 128), np.float32)
    p2[:64, :64] = pc
    p2[64:, 64:] = pc
    p2 = p2.astype(bf)

    k_idx = np.arange(128)[:, None]
    q_idx = np.arange(128)[None, :]
    md = (k_idx <= q_idx).astype(bf)   # diag block: valid k <= q
    mf = (k_idx > q_idx).astype(bf)    # far block: valid k > q
    ident = np.eye(128, dtype=np.float32).astype(bf)
    onesd = np.ones((128, 32), bf)
    return cs, sn, p2, md, mf, ident, onesd


_NC_CACHE = {}


def kernel(x, w_qkv, w_out):
    import ml_dtypes
    bf = ml_dtypes.bfloat16
    x = np.asarray(x, np.float32)
    w_qkv = np.asarray(w_qkv, np.float32)
    w_out = np.asarray(w_out, np.float32)
    B = x.shape[0]
    assert x.shape == (B, L, D) and B == 1

    if "nc" not in _NC_CACHE:
        _NC_CACHE["nc"] = _build_nc()
    nc = _NC_CACHE["nc"]

    xT = np.ascontiguousarray(x[0].T).astype(bf)       # [D, L]
    cs, sn, p2, md, mf, ident, onesd = _host_constants()

    in_maps = []
    for c in range(N_CORES):
        h0 = 2 * c
        col = slice(h0 * HD, (h0 + 2) * HD)
        wl = np.ascontiguousarray(np.concatenate(
            [w_qkv[:, 0 * D:1 * D][:, col],
             w_qkv[:, 1 * D:2 * D][:, col],
             w_qkv[:, 2 * D:3 * D][:, col]], axis=1)).astype(bf)  # [D, 384]
        wo = np.ascontiguousarray(
            w_out[h0 * HD:(h0 + 2) * HD, :]).astype(bf)  # [128, D]
        in_maps.append({"xT": xT, "wl": wl, "wo": wo, "p2": p2,
                        "cs": cs, "sn": sn, "md": md, "mf": mf,
                        "ident": ident, "onesd": onesd})

    res = run_bass_kernel_spmd(nc, in_maps, core_ids=list(range(N_CORES)))
    out = np.zeros((L, D), np.float64)
    for r in res.results:
        out += r["po"].astype(np.float64)
    return out.astype(np.float32)[None]
